# revision 16
# baseline (speedup 1.0000x reference)
"""Block-diagonal MLP kernel for Trainium2 (8 NeuronCores, data-parallel).

Computes out = blockdiag_matmul(x, weights) + bias where
  x: [4, 2048, 4096] f32, weights: [32, 128, 128] f32, bias: [4096] f32.

Strategy: shard the 8192 flattened batch rows across 8 cores (1024 rows
each), replicate weights/bias.  Per core, process 8 row-tiles of
[128, 4096]:
  - DMA x tile in (natural layout)
  - PE transpose-mode matmuls turn each [128,128] feature block into
    feature-major layout (contraction dim must be the partition dim)
  - fp32 matmuls against the resident weights
  - bias add fused into the PSUM->SBUF evacuation
  - DMA out tile
"""
import numpy as np
from contextlib import ExitStack

import concourse.bass as bass
import concourse.mybir as mybir
import concourse.tile as tile
from concourse import bacc
from concourse.bass_utils import run_bass_kernel_spmd
from concourse.masks import make_identity

F32 = mybir.dt.float32

SIZE = 4096
NB = 32          # number of diagonal blocks
BLK = 128        # block size
N_CORES = 8
B_FULL = 4 * 2048            # 8192 flattened rows
B_CORE = B_FULL // N_CORES   # 1024 rows per core
ROW_TILES = B_CORE // 128    # 8 tiles of 128 rows
GROUPS = SIZE // 512         # 8 groups of 4 blocks (512 cols) per row-tile

_NC_CACHE = {}


def _build_nc():
    nc = bacc.Bacc()
    x_d = nc.declare_dram_parameter("x", [B_CORE, SIZE], F32, isOutput=False)
    # weights pre-transposed on host to [d, k*128+e] so the SBUF tile
    # loads as one fully-contiguous transfer.
    w_d = nc.declare_dram_parameter("weights", [BLK, NB * BLK], F32, isOutput=False)
    b_d = nc.declare_dram_parameter("bias", [1, SIZE], F32, isOutput=False)
    o_d = nc.declare_dram_parameter("out", [B_CORE, SIZE], F32, isOutput=True)

    with tile.TileContext(nc) as tc, ExitStack() as ctx:
        consts = ctx.enter_context(tc.tile_pool(name="consts", bufs=1))
        x_pool = ctx.enter_context(tc.tile_pool(name="x", bufs=3))
        xt_pool = ctx.enter_context(tc.tile_pool(name="xt", bufs=4))
        out_pool = ctx.enter_context(tc.tile_pool(name="out", bufs=3))
        tp_pool = ctx.enter_context(tc.tile_pool(name="tp", bufs=3, space="PSUM"))
        mp_pool = ctx.enter_context(tc.tile_pool(name="mp", bufs=3, space="PSUM"))

        # Identity first (gpsimd, cheap) — needed by the very first transpose.
        ident = consts.tile([BLK, BLK], F32)
        make_identity(nc, ident)
        # Weights (host pre-transposed to d-major) in halves on the ACT
        # HWDGE ring so the first blocks' weights land early.
        w_sb = consts.tile([BLK, NB * BLK], F32)
        for h in range(2):
            cols = slice(h * 2048, (h + 1) * 2048)
            nc.scalar.dma_start(out=w_sb[:, cols], in_=w_d[:, cols])
        # Bias: replicate the 16 KiB DRAM row across partitions with 8
        # chunked broadcast DMAs on the SWDGE queue — trickles the re-read
        # so it doesn't contend with the early x/weight loads.
        bias_sb = consts.tile([128, SIZE], F32)
        b_ap = b_d[:, :]
        for g in range(GROUPS):
            src = bass.AP(
                tensor=b_ap.tensor,
                offset=b_ap.offset + g * 512,
                ap=[[0, 128], [1, 512]],
            )
            nc.gpsimd.dma_start(out=bias_sb[:, g * 512:(g + 1) * 512], in_=src)

        for t in range(ROW_TILES):
            x_tile = x_pool.tile([128, SIZE], F32)
            # Tile 0 loads in halves so the first transposes start sooner;
            # steady-state tiles load as one max-size transfer.
            if t == 0:
                for h in range(2):
                    nc.sync.dma_start(
                        out=x_tile[:, h * 2048:(h + 1) * 2048],
                        in_=x_d[t * 128:(t + 1) * 128, h * 2048:(h + 1) * 2048],
                    )
            else:
                nc.sync.dma_start(out=x_tile, in_=x_d[t * 128:(t + 1) * 128, :])
            out_tile = out_pool.tile([128, SIZE], F32)
            for g in range(GROUPS):
                # 4 transpose-mode matmuls into one PSUM bank: xT chunk
                tp = tp_pool.tile([128, 512], F32)
                for j in range(4):
                    k = 4 * g + j
                    nc.tensor.matmul(
                        tp[:, j * 128:(j + 1) * 128],
                        x_tile[:, k * 128:(k + 1) * 128],
                        ident,
                        is_transpose=True,
                        start=(j == 0),
                        stop=(j == 3),
                    )
                xt = xt_pool.tile([128, 512], F32)
                nc.scalar.copy(xt, tp)
                # 4 block matmuls into one PSUM bank: out chunk
                mp = mp_pool.tile([128, 512], F32)
                for j in range(4):
                    k = 4 * g + j
                    nc.tensor.matmul(
                        mp[:, j * 128:(j + 1) * 128],
                        xt[:, j * 128:(j + 1) * 128],
                        w_sb[:, k * 128:(k + 1) * 128],
                        start=(j == 0),
                        stop=(j == 3),
                    )
                # bias add fused into PSUM evacuation
                out_slice = out_tile[:, g * 512:(g + 1) * 512]
                bias_slice = bias_sb[:, g * 512:(g + 1) * 512]
                nc.vector.tensor_add(out_slice, mp, bias_slice)
            # Last tile stores in halves so the kernel tail only waits on
            # 1 MiB; steady-state tiles store as one max-size transfer.
            if t == ROW_TILES - 1:
                for h in range(2):
                    nc.scalar.dma_start(
                        out=o_d[t * 128:(t + 1) * 128, h * 2048:(h + 1) * 2048],
                        in_=out_tile[:, h * 2048:(h + 1) * 2048],
                    )
            else:
                nc.scalar.dma_start(out=o_d[t * 128:(t + 1) * 128, :], in_=out_tile)

    nc.compile()
    return nc


def _get_nc():
    if "nc" not in _NC_CACHE:
        _NC_CACHE["nc"] = _build_nc()
    return _NC_CACHE["nc"]


def _run(inputs, trace=False):
    x = np.asarray(inputs["x"], dtype=np.float32)
    weights = np.asarray(inputs["weights"], dtype=np.float32)
    bias = np.asarray(inputs["bias"], dtype=np.float32)
    orig_shape = x.shape
    xf = np.ascontiguousarray(x.reshape(B_FULL, SIZE))
    # Host-side layout for the small constants: weights d-major so the
    # SBUF tile loads contiguously, bias replicated across partitions.
    w_t = np.ascontiguousarray(
        weights.transpose(1, 0, 2).reshape(BLK, NB * BLK)
    )
    bias_row = np.ascontiguousarray(bias.reshape(1, SIZE))

    nc = _get_nc()
    in_maps = [
        {
            "x": xf[i * B_CORE:(i + 1) * B_CORE],
            "weights": w_t,
            "bias": bias_row,
        }
        for i in range(N_CORES)
    ]
    res = run_bass_kernel_spmd(
        nc, in_maps, core_ids=list(range(N_CORES)), trace=trace
    )
    out = np.concatenate([res.results[i]["out"] for i in range(N_CORES)], axis=0)
    return out.reshape(orig_shape), res


def kernel(**inputs):
    out, _ = _run(inputs, trace=False)
    return out


# revision 20
# speedup vs baseline: 1.1143x; 1.1143x over previous
"""Block-diagonal MLP kernel for Trainium2 (8 NeuronCores, data-parallel).

Computes out = blockdiag_matmul(x, weights) + bias where
  x: [4, 2048, 4096] f32, weights: [32, 128, 128] f32, bias: [4096] f32.

Strategy: shard the 8192 flattened batch rows across 8 cores (1024 rows
each), replicate weights/bias.  Per core, process 8 row-tiles of
[128, 4096]:
  - DMA x tile in (natural layout)
  - PE transpose-mode matmuls turn each [128,128] feature block into
    feature-major layout (contraction dim must be the partition dim)
  - fp32 matmuls against the resident weights
  - bias add fused into the PSUM->SBUF evacuation
  - DMA out tile
"""
import numpy as np
from contextlib import ExitStack

import concourse.bass as bass
import concourse.mybir as mybir
import concourse.tile as tile
from concourse import bacc
from concourse.bass_utils import run_bass_kernel_spmd
from concourse.masks import make_identity

F32 = mybir.dt.float32

SIZE = 4096
NB = 32          # number of diagonal blocks
BLK = 128        # block size
N_CORES = 8
B_FULL = 4 * 2048            # 8192 flattened rows
B_CORE = B_FULL // N_CORES   # 1024 rows per core
ROW_TILES = B_CORE // 128    # 8 tiles of 128 rows
GROUPS = SIZE // 512         # 8 groups of 4 blocks (512 cols) per row-tile

_NC_CACHE = {}


def _build_nc():
    nc = bacc.Bacc()
    x_d = nc.declare_dram_parameter("x", [B_CORE, SIZE], F32, isOutput=False)
    # weights pre-transposed on host to [d, k*128+e]; bias pre-replicated
    # to [128, SIZE] — both load as single fully-contiguous transfers.
    w_d = nc.declare_dram_parameter("weights", [BLK, NB * BLK], F32, isOutput=False)
    b_d = nc.declare_dram_parameter("bias", [128, SIZE], F32, isOutput=False)
    o_d = nc.declare_dram_parameter("out", [B_CORE, SIZE], F32, isOutput=True)

    with tile.TileContext(nc) as tc, ExitStack() as ctx:
        consts = ctx.enter_context(tc.tile_pool(name="consts", bufs=1))
        x_pool = ctx.enter_context(tc.tile_pool(name="x", bufs=3))
        xt_pool = ctx.enter_context(tc.tile_pool(name="xt", bufs=4))
        out_pool = ctx.enter_context(tc.tile_pool(name="out", bufs=3))
        tp_pool = ctx.enter_context(tc.tile_pool(name="tp", bufs=3, space="PSUM"))
        mp_pool = ctx.enter_context(tc.tile_pool(name="mp", bufs=4, space="PSUM"))

        # Identity first (gpsimd, cheap) — needed by the very first transpose.
        ident = consts.tile([BLK, BLK], F32)
        make_identity(nc, ident)
        # Weights (host pre-transposed to d-major) then bias (host
        # pre-replicated), each one fully-contiguous 2 MiB transfer on the
        # ACT HWDGE ring.
        w_sb = consts.tile([BLK, NB * BLK], F32)
        nc.scalar.dma_start(out=w_sb, in_=w_d[:, :])
        bias_sb = consts.tile([128, SIZE], F32)
        nc.scalar.dma_start(out=bias_sb, in_=b_d[:, :])

        for t in range(ROW_TILES):
            x_tile = x_pool.tile([128, SIZE], F32)
            # Tile 0 loads in halves so the first transposes start sooner;
            # steady-state tiles load as one max-size transfer.
            if t == 0:
                for h in range(2):
                    nc.sync.dma_start(
                        out=x_tile[:, h * 2048:(h + 1) * 2048],
                        in_=x_d[t * 128:(t + 1) * 128, h * 2048:(h + 1) * 2048],
                    )
            else:
                nc.sync.dma_start(out=x_tile, in_=x_d[t * 128:(t + 1) * 128, :])
            out_tile = out_pool.tile([128, SIZE], F32)
            for g in range(GROUPS):
                # 4 transpose-mode matmuls into one PSUM bank: xT chunk
                tp = tp_pool.tile([128, 512], F32)
                for j in range(4):
                    k = 4 * g + j
                    nc.tensor.matmul(
                        tp[:, j * 128:(j + 1) * 128],
                        x_tile[:, k * 128:(k + 1) * 128],
                        ident,
                        is_transpose=True,
                        start=(j == 0),
                        stop=(j == 3),
                    )
                xt = xt_pool.tile([128, 512], F32)
                nc.scalar.copy(xt, tp)
                # 4 block matmuls into one PSUM bank: out chunk
                mp = mp_pool.tile([128, 512], F32)
                for j in range(4):
                    k = 4 * g + j
                    nc.tensor.matmul(
                        mp[:, j * 128:(j + 1) * 128],
                        xt[:, j * 128:(j + 1) * 128],
                        w_sb[:, k * 128:(k + 1) * 128],
                        start=(j == 0),
                        stop=(j == 3),
                    )
                # bias add fused into PSUM evacuation
                out_slice = out_tile[:, g * 512:(g + 1) * 512]
                bias_slice = bias_sb[:, g * 512:(g + 1) * 512]
                nc.vector.tensor_add(out_slice, mp, bias_slice)
            # Last tile stores in halves so the kernel tail only waits on
            # 1 MiB; steady-state tiles store as one max-size transfer.
            if t == ROW_TILES - 1:
                for h in range(2):
                    nc.scalar.dma_start(
                        out=o_d[t * 128:(t + 1) * 128, h * 2048:(h + 1) * 2048],
                        in_=out_tile[:, h * 2048:(h + 1) * 2048],
                    )
            else:
                nc.scalar.dma_start(out=o_d[t * 128:(t + 1) * 128, :], in_=out_tile)

    nc.compile()
    return nc


def _get_nc():
    if "nc" not in _NC_CACHE:
        _NC_CACHE["nc"] = _build_nc()
    return _NC_CACHE["nc"]


def _run(inputs, trace=False):
    x = np.asarray(inputs["x"], dtype=np.float32)
    weights = np.asarray(inputs["weights"], dtype=np.float32)
    bias = np.asarray(inputs["bias"], dtype=np.float32)
    orig_shape = x.shape
    xf = np.ascontiguousarray(x.reshape(B_FULL, SIZE))
    # Host-side layout for the small constants: weights d-major so the
    # SBUF tile loads contiguously, bias replicated across partitions.
    w_t = np.ascontiguousarray(
        weights.transpose(1, 0, 2).reshape(BLK, NB * BLK)
    )
    bias_rep = np.ascontiguousarray(np.broadcast_to(bias[None, :], (128, SIZE)))

    nc = _get_nc()
    in_maps = [
        {
            "x": xf[i * B_CORE:(i + 1) * B_CORE],
            "weights": w_t,
            "bias": bias_rep,
        }
        for i in range(N_CORES)
    ]
    res = run_bass_kernel_spmd(
        nc, in_maps, core_ids=list(range(N_CORES)), trace=trace
    )
    out = np.concatenate([res.results[i]["out"] for i in range(N_CORES)], axis=0)
    return out.reshape(orig_shape), res


def kernel(**inputs):
    out, _ = _run(inputs, trace=False)
    return out


# revision 22
# speedup vs baseline: 1.1364x; 1.0199x over previous
"""Block-diagonal MLP kernel for Trainium2 (8 NeuronCores, data-parallel).

Computes out = blockdiag_matmul(x, weights) + bias where
  x: [4, 2048, 4096] f32, weights: [32, 128, 128] f32, bias: [4096] f32.

Strategy: shard the 8192 flattened batch rows across 8 cores (1024 rows
each), replicate weights/bias.  Per core, process 8 row-tiles of
[128, 4096]:
  - DMA x tile in (natural layout)
  - PE transpose-mode matmuls turn each [128,128] feature block into
    feature-major layout (contraction dim must be the partition dim)
  - fp32 matmuls against the resident weights
  - bias add fused into the PSUM->SBUF evacuation
  - DMA out tile
"""
import numpy as np
from contextlib import ExitStack

import concourse.bass as bass
import concourse.mybir as mybir
import concourse.tile as tile
from concourse import bacc
from concourse.bass_utils import run_bass_kernel_spmd
from concourse.masks import make_identity

F32 = mybir.dt.float32

SIZE = 4096
NB = 32          # number of diagonal blocks
BLK = 128        # block size
N_CORES = 8
B_FULL = 4 * 2048            # 8192 flattened rows
B_CORE = B_FULL // N_CORES   # 1024 rows per core
ROW_TILES = B_CORE // 128    # 8 tiles of 128 rows
GROUPS = SIZE // 512         # 8 groups of 4 blocks (512 cols) per row-tile

_NC_CACHE = {}


def _build_nc():
    nc = bacc.Bacc()
    x_d = nc.declare_dram_parameter("x", [B_CORE, SIZE], F32, isOutput=False)
    # weights pre-transposed on host to [d, k*128+e]; bias pre-replicated
    # to [128, SIZE] — both load as single fully-contiguous transfers.
    w_d = nc.declare_dram_parameter("weights", [BLK, NB * BLK], F32, isOutput=False)
    b_d = nc.declare_dram_parameter("bias", [128, SIZE], F32, isOutput=False)
    o_d = nc.declare_dram_parameter("out", [B_CORE, SIZE], F32, isOutput=True)

    with tile.TileContext(nc) as tc, ExitStack() as ctx:
        consts = ctx.enter_context(tc.tile_pool(name="consts", bufs=1))
        x_pool = ctx.enter_context(tc.tile_pool(name="x", bufs=3))
        xt_pool = ctx.enter_context(tc.tile_pool(name="xt", bufs=4))
        out_pool = ctx.enter_context(tc.tile_pool(name="out", bufs=3))
        tp_pool = ctx.enter_context(tc.tile_pool(name="tp", bufs=3, space="PSUM"))
        mp_pool = ctx.enter_context(tc.tile_pool(name="mp", bufs=4, space="PSUM"))

        # Identity first (gpsimd, cheap) — needed by the very first transpose.
        ident = consts.tile([BLK, BLK], F32)
        make_identity(nc, ident)
        # Weights (host pre-transposed to d-major) then bias (host
        # pre-replicated), each one fully-contiguous 2 MiB transfer on the
        # ACT HWDGE ring.
        w_sb = consts.tile([BLK, NB * BLK], F32)
        nc.scalar.dma_start(out=w_sb, in_=w_d[:, :])
        bias_sb = consts.tile([128, SIZE], F32)
        nc.scalar.dma_start(out=bias_sb, in_=b_d[:, :])

        for t in range(ROW_TILES):
            x_tile = x_pool.tile([128, SIZE], F32)
            # Tile 0 loads a small first chunk so the first transposes start
            # sooner; steady-state tiles load as one max-size transfer.
            if t == 0:
                nc.sync.dma_start(
                    out=x_tile[:, 0:512], in_=x_d[0:128, 0:512]
                )
                nc.sync.dma_start(
                    out=x_tile[:, 512:SIZE], in_=x_d[0:128, 512:SIZE]
                )
            else:
                nc.sync.dma_start(out=x_tile, in_=x_d[t * 128:(t + 1) * 128, :])
            out_tile = out_pool.tile([128, SIZE], F32)
            for g in range(GROUPS):
                # 4 transpose-mode matmuls into one PSUM bank: xT chunk
                tp = tp_pool.tile([128, 512], F32)
                for j in range(4):
                    k = 4 * g + j
                    nc.tensor.matmul(
                        tp[:, j * 128:(j + 1) * 128],
                        x_tile[:, k * 128:(k + 1) * 128],
                        ident,
                        is_transpose=True,
                        start=(j == 0),
                        stop=(j == 3),
                    )
                xt = xt_pool.tile([128, 512], F32)
                nc.scalar.copy(xt, tp)
                # 4 block matmuls into one PSUM bank: out chunk
                mp = mp_pool.tile([128, 512], F32)
                for j in range(4):
                    k = 4 * g + j
                    nc.tensor.matmul(
                        mp[:, j * 128:(j + 1) * 128],
                        xt[:, j * 128:(j + 1) * 128],
                        w_sb[:, k * 128:(k + 1) * 128],
                        start=(j == 0),
                        stop=(j == 3),
                    )
                # bias add fused into PSUM evacuation
                out_slice = out_tile[:, g * 512:(g + 1) * 512]
                bias_slice = bias_sb[:, g * 512:(g + 1) * 512]
                nc.vector.tensor_add(out_slice, mp, bias_slice)
            # Stores alternate between the two HWDGE rings so the final
            # stores don't serialize behind each other; the last tile goes
            # out in quarters so the kernel tail only waits on 256 KiB.
            rows = slice(t * 128, (t + 1) * 128)
            if t == ROW_TILES - 1:
                for q in range(4):
                    eng = nc.scalar if q % 2 == 0 else nc.sync
                    cols = slice(q * 1024, (q + 1) * 1024)
                    eng.dma_start(out=o_d[rows, cols], in_=out_tile[:, cols])
            else:
                eng = nc.scalar if t % 2 == 0 else nc.sync
                eng.dma_start(out=o_d[rows, :], in_=out_tile)

    nc.compile()
    return nc


def _get_nc():
    if "nc" not in _NC_CACHE:
        _NC_CACHE["nc"] = _build_nc()
    return _NC_CACHE["nc"]


def _run(inputs, trace=False):
    x = np.asarray(inputs["x"], dtype=np.float32)
    weights = np.asarray(inputs["weights"], dtype=np.float32)
    bias = np.asarray(inputs["bias"], dtype=np.float32)
    orig_shape = x.shape
    xf = np.ascontiguousarray(x.reshape(B_FULL, SIZE))
    # Host-side layout for the small constants: weights d-major so the
    # SBUF tile loads contiguously, bias replicated across partitions.
    w_t = np.ascontiguousarray(
        weights.transpose(1, 0, 2).reshape(BLK, NB * BLK)
    )
    bias_rep = np.ascontiguousarray(np.broadcast_to(bias[None, :], (128, SIZE)))

    nc = _get_nc()
    in_maps = [
        {
            "x": xf[i * B_CORE:(i + 1) * B_CORE],
            "weights": w_t,
            "bias": bias_rep,
        }
        for i in range(N_CORES)
    ]
    res = run_bass_kernel_spmd(
        nc, in_maps, core_ids=list(range(N_CORES)), trace=trace
    )
    out = np.concatenate([res.results[i]["out"] for i in range(N_CORES)], axis=0)
    return out.reshape(orig_shape), res


def kernel(**inputs):
    out, _ = _run(inputs, trace=False)
    return out
